# revision 13
# baseline (speedup 1.0000x reference)
"""Trainium2 Bass kernel for nn_ChannelAttention (squeeze-excite).

Reference computation:
    s = mean(x, axis=(H, W))                    # [B, C]   global avg pool
    h = relu(bn1(s @ w1))                       # [B, Cr]  Cr = 16
    o = bn2(h @ w2)                             # [B, C]
    return o[:, None, None, :]                  # [B, 1, 1, C]

Strategy (data-parallel over batch, 8 cores x 8 samples):
  - Each core streams its 8 samples (3.2 MB each, one HWDGE DMA per sample)
    into SBUF tiles of [112 partitions, 7168] (28 spatial rows per partition,
    channel-aligned since 28*256 = 7168).
  - Squeeze: partition-axis reduction via TensorE with a ones[112,1] lhsT,
    accumulating per-sample channel sums into PSUM ([1,512] per sample; the
    512 free dim holds channel c in both col c and c+256, folded later).
  - Per-sample sums are copied to SBUF, folded to [1,256], then gathered into
    an sT layout [128ch, 8samples] x2 via K=1 one-hot matmuls (this doubles
    as the transpose needed for the excite MLP).
  - Excite MLP on PE: g1[16,8] = w1.T @ sT (K=256 split in 2), BN1+ReLU as a
    single ScalarE activation (per-partition scale/bias APs, with the 1/HW
    mean scale folded into BN1's scale), o[8,256] = h.T @ w2, BN2 applied
    with parameters broadcast to [8,256] via zero-stride DMA.
"""

import sys

if "/opt/trn_rl_repo" not in sys.path:
    sys.path.insert(0, "/opt/trn_rl_repo")

import numpy as np

B, H, W, C = 64, 56, 56, 256
CR = 16
NCORES = 8
BL = B // NCORES  # samples per core
HWP = H * W  # 3136 spatial positions
P = 112  # partitions per x tile (3136 = 112 * 28)
RPP = HWP // P  # 28 rows per partition
FD = RPP * C  # 7168 free-dim elements per partition
NK = FD // 512  # 14 matmul column slices
EPS = 1e-3

_CACHE: dict = {}


def _build_nc():
    import concourse.bass as bass
    import concourse.tile as tile
    from concourse import bacc, mybir
    from contextlib import ExitStack

    f32 = mybir.dt.float32
    AF = mybir.ActivationFunctionType

    nc = bacc.Bacc("TRN2", target_bir_lowering=False, debug=False)

    x_d = nc.dram_tensor("x", [BL, P, FD], f32, kind="ExternalInput")
    w1_d = nc.dram_tensor("w1", [C, CR], f32, kind="ExternalInput")
    ga1_d = nc.dram_tensor("gamma1", [CR, 1], f32, kind="ExternalInput")
    be1_d = nc.dram_tensor("beta1", [CR, 1], f32, kind="ExternalInput")
    mu1_d = nc.dram_tensor("mean1", [CR, 1], f32, kind="ExternalInput")
    va1_d = nc.dram_tensor("var1", [CR, 1], f32, kind="ExternalInput")
    w2_d = nc.dram_tensor("w2", [CR, C], f32, kind="ExternalInput")
    ga2_d = nc.dram_tensor("gamma2", [C], f32, kind="ExternalInput")
    be2_d = nc.dram_tensor("beta2", [C], f32, kind="ExternalInput")
    mu2_d = nc.dram_tensor("mean2", [C], f32, kind="ExternalInput")
    va2_d = nc.dram_tensor("var2", [C], f32, kind="ExternalInput")
    out_d = nc.dram_tensor("out", [BL, C], f32, kind="ExternalOutput")

    def bcast(d):
        # [C] dram vector -> [BL, C] read AP with zero partition stride
        a = d[:]
        return bass.AP(tensor=a.tensor, offset=a.offset, ap=[[0, BL], [1, C]])

    with ExitStack() as ctx:
        tc = ctx.enter_context(tile.TileContext(nc))
        xp = ctx.enter_context(tc.tile_pool(name="xp", bufs=4))
        pp = ctx.enter_context(tc.tile_pool(name="pp", bufs=1))
        accp = ctx.enter_context(tc.tile_pool(name="accp", bufs=4, space="PSUM"))
        mlpp = ctx.enter_context(tc.tile_pool(name="mlpp", bufs=1, space="PSUM"))

        # ---- constants / parameters (all overlap with the main stream) ----
        ones_t = pp.tile([P, 1], f32, tag="ones", name="ones_t")
        nc.vector.memset(ones_t, 1.0)

        # one-hot bank: oh[p, b, j] = (b == j), identical on every partition
        oh = pp.tile([128, BL, BL], f32, tag="oh", name="oh")
        nc.vector.memset(oh, 0.0)
        for b in range(BL):
            nc.vector.memset(oh[:, b, b : b + 1], 1.0)

        w1a = pp.tile([128, CR], f32, tag="w1a", name="w1a")
        nc.sync.dma_start(w1a, w1_d[0:128, :])
        w1b = pp.tile([128, CR], f32, tag="w1b", name="w1b")
        nc.sync.dma_start(w1b, w1_d[128:256, :])
        w2t = pp.tile([CR, C], f32, tag="w2t", name="w2t")
        nc.sync.dma_start(w2t, w2_d[:, :])

        # BN1 parameters, [16, 1] per-partition layout
        ga1 = pp.tile([CR, 1], f32, tag="ga1", name="ga1")
        nc.sync.dma_start(ga1, ga1_d[:, :])
        be1 = pp.tile([CR, 1], f32, tag="be1", name="be1")
        nc.sync.dma_start(be1, be1_d[:, :])
        mu1 = pp.tile([CR, 1], f32, tag="mu1", name="mu1")
        nc.sync.dma_start(mu1, mu1_d[:, :])
        va1 = pp.tile([CR, 1], f32, tag="va1", name="va1")
        nc.sync.dma_start(va1, va1_d[:, :])

        # scale1 = gamma1 / sqrt(var1 + eps) / HW, bias1 = beta1 - mean1 * k1
        # (route activation deps through a single engine: the Activation
        # instruction encoding only has room for one sync wait when bias
        # is an AP, so both of its inputs must come from the same sem)
        eps1 = pp.tile([CR, 1], f32, tag="eps1", name="eps1")
        nc.vector.memset(eps1, EPS)
        va1c = pp.tile([CR, 1], f32, tag="va1c", name="va1c")
        nc.vector.tensor_copy(va1c, va1)
        srt1 = pp.tile([CR, 1], f32, tag="srt1", name="srt1")
        nc.scalar.activation(srt1, va1c, AF.Sqrt, bias=eps1)
        rst1 = pp.tile([CR, 1], f32, tag="rst1", name="rst1")
        nc.vector.reciprocal(rst1, srt1)
        k1 = pp.tile([CR, 1], f32, tag="k1", name="k1")
        nc.vector.tensor_mul(k1, ga1, rst1)
        sc1 = pp.tile([CR, 1], f32, tag="sc1", name="sc1")
        nc.scalar.mul(sc1, k1, 1.0 / HWP)
        tm1 = pp.tile([CR, 1], f32, tag="tm1", name="tm1")
        nc.vector.tensor_mul(tm1, mu1, k1)
        bi1 = pp.tile([CR, 1], f32, tag="bi1", name="bi1")
        nc.vector.tensor_sub(bi1, be1, tm1)

        # BN2 parameters broadcast to [BL, C]
        ga2 = pp.tile([BL, C], f32, tag="ga2", name="ga2")
        nc.gpsimd.dma_start(ga2, bcast(ga2_d))
        be2 = pp.tile([BL, C], f32, tag="be2", name="be2")
        nc.gpsimd.dma_start(be2, bcast(be2_d))
        mu2 = pp.tile([BL, C], f32, tag="mu2", name="mu2")
        nc.gpsimd.dma_start(mu2, bcast(mu2_d))
        va2 = pp.tile([BL, C], f32, tag="va2", name="va2")
        nc.gpsimd.dma_start(va2, bcast(va2_d))

        eps2 = pp.tile([BL, 1], f32, tag="eps2", name="eps2")
        nc.vector.memset(eps2, EPS)
        va2c = pp.tile([BL, C], f32, tag="va2c", name="va2c")
        nc.vector.tensor_copy(va2c, va2)
        srt2 = pp.tile([BL, C], f32, tag="srt2", name="srt2")
        nc.scalar.activation(srt2, va2c, AF.Sqrt, bias=eps2)
        rst2 = pp.tile([BL, C], f32, tag="rst2", name="rst2")
        nc.vector.reciprocal(rst2, srt2)
        k2 = pp.tile([BL, C], f32, tag="k2", name="k2")
        nc.vector.tensor_mul(k2, ga2, rst2)
        tm2 = pp.tile([BL, C], f32, tag="tm2", name="tm2")
        nc.vector.tensor_mul(tm2, mu2, k2)
        bi2 = pp.tile([BL, C], f32, tag="bi2", name="bi2")
        nc.vector.tensor_sub(bi2, be2, tm2)

        # ---- stage 1: squeeze (global sum over H*W per sample/channel) ----
        # acc_sb[p, q, :]: raw [1, 512] channel sums for sample 2q + p/32
        acc_sb = pp.tile([128, 4, 512], f32, tag="acc_sb", name="acc_sb")
        # s_sb[p, q, :]: folded [1, 256] sums
        s_sb = pp.tile([128, 4, C], f32, tag="s_sb", name="s_sb")
        # sT[c, b] per channel half -> MLP rhs
        sT0 = mlpp.tile([128, BL], f32, tag="sT0", name="sT0")
        sT1 = mlpp.tile([128, BL], f32, tag="sT1", name="sT1")

        for q in range(4):
            acc = accp.tile([128, 512], f32, tag="acc", name=f"acc{q}")
            for j in range(2):
                b = 2 * q + j
                pb = 32 * j
                xt = xp.tile([P, FD], f32, tag="xt", name=f"xt{b}")
                nc.sync.dma_start(xt, x_d[b])
                for k in range(NK):
                    nc.tensor.matmul(
                        acc[pb : pb + 1, :],
                        ones_t,
                        xt[:, k * 512 : (k + 1) * 512],
                        start=(k == 0),
                        stop=(k == NK - 1),
                    )
            for j in range(2):
                pb = 32 * j
                nc.scalar.copy(acc_sb[pb : pb + 1, q, :], acc[pb : pb + 1, :])
                nc.vector.tensor_add(
                    s_sb[pb : pb + 1, q, :],
                    acc_sb[pb : pb + 1, q, 0:C],
                    acc_sb[pb : pb + 1, q, C : 2 * C],
                )
            # gather this pair's sums into the sT layout (also the transpose)
            for j in range(2):
                b = 2 * q + j
                pb = 32 * j
                for h, sT in enumerate((sT0, sT1)):
                    nc.tensor.matmul(
                        sT[:, 0:BL],
                        s_sb[pb : pb + 1, q, h * 128 : (h + 1) * 128],
                        oh[pb : pb + 1, b, :],
                        start=(b == 0),
                        stop=(b == BL - 1),
                    )

        # ---- stage 2: excite MLP ----
        sT0s = pp.tile([128, BL], f32, tag="sT0s", name="sT0s")
        nc.scalar.copy(sT0s, sT0)
        sT1s = pp.tile([128, BL], f32, tag="sT1s", name="sT1s")
        nc.vector.tensor_copy(sT1s, sT1)

        g1p = mlpp.tile([CR, BL], f32, tag="g1p", name="g1p")
        nc.tensor.matmul(g1p, w1a, sT0s, start=True, stop=False)
        nc.tensor.matmul(g1p, w1b, sT1s, start=False, stop=True)

        # h = relu(g1 * scale1 + bias1)  (BN1 + mean scale + relu in one op).
        # bi1 comes from DVE; copy it through ACT so the Relu activation's
        # only cross-engine wait is on the PE matmul result.
        bi1c = pp.tile([CR, 1], f32, tag="bi1c", name="bi1c")
        nc.scalar.copy(bi1c, bi1)
        sc1c = pp.tile([CR, 1], f32, tag="sc1c", name="sc1c")
        nc.scalar.copy(sc1c, sc1)
        h_sb = pp.tile([CR, BL], f32, tag="h_sb", name="h_sb")
        nc.scalar.activation(h_sb, g1p, AF.Relu, bias=bi1c, scale=sc1c)

        o_p = mlpp.tile([BL, C], f32, tag="o_p", name="o_p")
        nc.tensor.matmul(o_p, h_sb, w2t, start=True, stop=True)

        osc = pp.tile([BL, C], f32, tag="osc", name="osc")
        nc.vector.tensor_mul(osc, o_p, k2)
        ofin = pp.tile([BL, C], f32, tag="ofin", name="ofin")
        nc.vector.tensor_add(ofin, osc, bi2)
        nc.sync.dma_start(out_d[:, :], ofin)

    nc.compile()
    return nc


def _get_nc():
    if "nc" not in _CACHE:
        _CACHE["nc"] = _build_nc()
    return _CACHE["nc"]


def _in_maps(inputs):
    x = np.ascontiguousarray(np.asarray(inputs["x"], dtype=np.float32))
    w1 = np.ascontiguousarray(np.asarray(inputs["w1"], dtype=np.float32))
    w2 = np.ascontiguousarray(np.asarray(inputs["w2"], dtype=np.float32))
    p1 = {
        k: np.ascontiguousarray(
            np.asarray(inputs[k], dtype=np.float32).reshape(CR, 1)
        )
        for k in ("gamma1", "beta1", "mean1", "var1")
    }
    p2 = {
        k: np.ascontiguousarray(np.asarray(inputs[k], dtype=np.float32).reshape(C))
        for k in ("gamma2", "beta2", "mean2", "var2")
    }
    maps = []
    for c in range(NCORES):
        shard = np.ascontiguousarray(x[c * BL : (c + 1) * BL]).reshape(BL, P, FD)
        maps.append({"x": shard, "w1": w1, "w2": w2, **p1, **p2})
    return maps


def _run(inputs, trace=False):
    from concourse.bass_utils import run_bass_kernel_spmd

    nc = _get_nc()
    res = run_bass_kernel_spmd(
        nc, _in_maps(inputs), core_ids=list(range(NCORES)), trace=trace
    )
    out = np.concatenate([res.results[c]["out"] for c in range(NCORES)], axis=0)
    return out.reshape(B, 1, 1, C).astype(np.float32), res


def kernel(**inputs) -> np.ndarray:
    out, _ = _run(inputs, trace=False)
    return out


def kernel_traced(**inputs):
    """Returns (out, BassKernelResults) with NTFF profiling enabled."""
    return _run(inputs, trace=True)


def bench(inputs, iters=30, warmup=5):
    """Time the per-step NEFF execution with device-resident inputs.

    Returns (out_full, per_call_seconds_list). Inputs are device_put once;
    each timed call only dispatches the compiled executable, so steady-state
    per-call wall time ~= max-core NEFF exec + dispatch overhead.
    """
    import time
    import jax
    import jax.numpy as jnp
    from jax.sharding import Mesh, PartitionSpec, NamedSharding
    from jax.experimental.shard_map import shard_map
    from concourse import bass2jax, mybir

    bass2jax.install_neuronx_cc_hook()
    nc = _get_nc()

    partition_name = nc.partition_id_tensor.name if nc.partition_id_tensor else None
    in_names, out_names, out_avals = [], [], []
    for alloc in nc.m.functions[0].allocations:
        if not isinstance(alloc, mybir.MemoryLocationSet):
            continue
        name = alloc.memorylocations[0].name
        if alloc.kind == "ExternalInput":
            if name != partition_name:
                in_names.append(name)
        elif alloc.kind == "ExternalOutput":
            out_names.append(name)
            out_avals.append(
                jax.core.ShapedArray(tuple(alloc.tensor_shape), mybir.dt.np(alloc.dtype))
            )
    all_in_names = in_names + out_names
    if partition_name is not None:
        all_in_names = all_in_names + [partition_name]

    def _body(*operands):
        operands = list(operands)
        if partition_name is not None:
            operands.append(bass2jax.partition_id_tensor())
        outs = bass2jax._bass_exec_p.bind(
            *operands,
            out_avals=tuple(out_avals),
            in_names=tuple(all_in_names),
            out_names=tuple(out_names),
            lowering_input_output_aliases=(),
            sim_require_finite=True,
            sim_require_nnan=True,
            nc=nc,
        )
        return tuple(outs)

    devices = jax.devices()[:NCORES]
    mesh = Mesh(np.asarray(devices), ("core",))
    spec = PartitionSpec("core")
    maps = _in_maps(inputs)
    concat = [
        np.concatenate([maps[c][n] for c in range(NCORES)], axis=0) for n in in_names
    ]
    concat += [
        np.zeros((NCORES * a.shape[0], *a.shape[1:]), a.dtype) for a in out_avals
    ]
    sharding = NamedSharding(mesh, spec)
    dev_in = [jax.device_put(a, sharding) for a in concat]

    fn = jax.jit(
        shard_map(
            _body,
            mesh=mesh,
            in_specs=(spec,) * len(concat),
            out_specs=(spec,) * len(out_names),
            check_rep=False,
        )
    )

    for _ in range(warmup):
        outs = fn(*dev_in)
    jax.block_until_ready(outs)

    times = []
    for _ in range(iters):
        t0 = time.perf_counter()
        outs = fn(*dev_in)
        jax.block_until_ready(outs)
        times.append(time.perf_counter() - t0)

    oidx = out_names.index("out")
    o = np.asarray(outs[oidx]).reshape(NCORES, BL, C).reshape(B, C)
    return o.reshape(B, 1, 1, C).astype(np.float32), times


# revision 14
# speedup vs baseline: 1.0388x; 1.0388x over previous
"""Trainium2 Bass kernel for nn_ChannelAttention (squeeze-excite).

Reference computation:
    s = mean(x, axis=(H, W))                    # [B, C]   global avg pool
    h = relu(bn1(s @ w1))                       # [B, Cr]  Cr = 16
    o = bn2(h @ w2)                             # [B, C]
    return o[:, None, None, :]                  # [B, 1, 1, C]

Strategy (data-parallel over batch, 8 cores x 8 samples):
  - Each core streams its 8 samples (3.2 MB each, one HWDGE DMA per sample)
    into SBUF tiles of [112 partitions, 7168] (28 spatial rows per partition,
    channel-aligned since 28*256 = 7168).
  - Squeeze: partition-axis reduction via TensorE with a ones[112,1] lhsT,
    accumulating per-sample channel sums into PSUM ([1,512] per sample; the
    512 free dim holds channel c in both col c and c+256, folded later).
  - Per-sample sums are copied to SBUF, folded to [1,256], then gathered into
    an sT layout [128ch, 8samples] x2 via K=1 one-hot matmuls (this doubles
    as the transpose needed for the excite MLP).
  - Excite MLP on PE: g1[16,8] = w1.T @ sT (K=256 split in 2), BN1+ReLU as a
    single ScalarE activation (per-partition scale/bias APs, with the 1/HW
    mean scale folded into BN1's scale), o[8,256] = h.T @ w2, BN2 applied
    with parameters broadcast to [8,256] via zero-stride DMA.
"""

import sys

if "/opt/trn_rl_repo" not in sys.path:
    sys.path.insert(0, "/opt/trn_rl_repo")

import numpy as np

B, H, W, C = 64, 56, 56, 256
CR = 16
NCORES = 8
BL = B // NCORES  # samples per core
HWP = H * W  # 3136 spatial positions
P = 112  # partitions per x tile (3136 = 112 * 28)
RPP = HWP // P  # 28 rows per partition
FD = RPP * C  # 7168 free-dim elements per partition
NK = FD // 512  # 14 matmul column slices
EPS = 1e-3

_CACHE: dict = {}


def _build_nc():
    import concourse.bass as bass
    import concourse.tile as tile
    from concourse import bacc, mybir
    from contextlib import ExitStack

    f32 = mybir.dt.float32
    AF = mybir.ActivationFunctionType

    nc = bacc.Bacc("TRN2", target_bir_lowering=False, debug=False)

    x_d = nc.dram_tensor("x", [BL, P, FD], f32, kind="ExternalInput")
    w1_d = nc.dram_tensor("w1", [C, CR], f32, kind="ExternalInput")
    ga1_d = nc.dram_tensor("gamma1", [CR, 1], f32, kind="ExternalInput")
    be1_d = nc.dram_tensor("beta1", [CR, 1], f32, kind="ExternalInput")
    mu1_d = nc.dram_tensor("mean1", [CR, 1], f32, kind="ExternalInput")
    va1_d = nc.dram_tensor("var1", [CR, 1], f32, kind="ExternalInput")
    w2_d = nc.dram_tensor("w2", [CR, C], f32, kind="ExternalInput")
    ga2_d = nc.dram_tensor("gamma2", [C], f32, kind="ExternalInput")
    be2_d = nc.dram_tensor("beta2", [C], f32, kind="ExternalInput")
    mu2_d = nc.dram_tensor("mean2", [C], f32, kind="ExternalInput")
    va2_d = nc.dram_tensor("var2", [C], f32, kind="ExternalInput")
    out_d = nc.dram_tensor("out", [BL, C], f32, kind="ExternalOutput")

    def bcast(d):
        # [C] dram vector -> [BL, C] read AP with zero partition stride
        a = d[:]
        return bass.AP(tensor=a.tensor, offset=a.offset, ap=[[0, BL], [1, C]])

    with ExitStack() as ctx:
        tc = ctx.enter_context(tile.TileContext(nc))
        xp = ctx.enter_context(tc.tile_pool(name="xp", bufs=4))
        pp = ctx.enter_context(tc.tile_pool(name="pp", bufs=1))
        accp = ctx.enter_context(tc.tile_pool(name="accp", bufs=4, space="PSUM"))
        mlpp = ctx.enter_context(tc.tile_pool(name="mlpp", bufs=1, space="PSUM"))

        # ---- constants / parameters (all overlap with the main stream) ----
        ones_t = pp.tile([P, 1], f32, tag="ones", name="ones_t")
        nc.vector.memset(ones_t, 1.0)

        # one-hot bank: oh[p, b, j] = (b == j), identical on every partition
        oh = pp.tile([128, BL, BL], f32, tag="oh", name="oh")
        nc.vector.memset(oh, 0.0)
        for b in range(BL):
            nc.vector.memset(oh[:, b, b : b + 1], 1.0)

        w1a = pp.tile([128, CR], f32, tag="w1a", name="w1a")
        nc.sync.dma_start(w1a, w1_d[0:128, :])
        w1b = pp.tile([128, CR], f32, tag="w1b", name="w1b")
        nc.sync.dma_start(w1b, w1_d[128:256, :])
        w2t = pp.tile([CR, C], f32, tag="w2t", name="w2t")
        nc.sync.dma_start(w2t, w2_d[:, :])

        # BN1 parameters, [16, 1] per-partition layout
        ga1 = pp.tile([CR, 1], f32, tag="ga1", name="ga1")
        nc.sync.dma_start(ga1, ga1_d[:, :])
        be1 = pp.tile([CR, 1], f32, tag="be1", name="be1")
        nc.sync.dma_start(be1, be1_d[:, :])
        mu1 = pp.tile([CR, 1], f32, tag="mu1", name="mu1")
        nc.sync.dma_start(mu1, mu1_d[:, :])
        va1 = pp.tile([CR, 1], f32, tag="va1", name="va1")
        nc.sync.dma_start(va1, va1_d[:, :])

        # scale1 = gamma1 / sqrt(var1 + eps) / HW, bias1 = beta1 - mean1 * k1
        # (route activation deps through a single engine: the Activation
        # instruction encoding only has room for one sync wait when bias
        # is an AP, so both of its inputs must come from the same sem)
        eps1 = pp.tile([CR, 1], f32, tag="eps1", name="eps1")
        nc.vector.memset(eps1, EPS)
        va1c = pp.tile([CR, 1], f32, tag="va1c", name="va1c")
        nc.vector.tensor_copy(va1c, va1)
        srt1 = pp.tile([CR, 1], f32, tag="srt1", name="srt1")
        nc.scalar.activation(srt1, va1c, AF.Sqrt, bias=eps1)
        rst1 = pp.tile([CR, 1], f32, tag="rst1", name="rst1")
        nc.vector.reciprocal(rst1, srt1)
        k1 = pp.tile([CR, 1], f32, tag="k1", name="k1")
        nc.vector.tensor_mul(k1, ga1, rst1)
        sc1 = pp.tile([CR, 1], f32, tag="sc1", name="sc1")
        nc.scalar.mul(sc1, k1, 1.0 / HWP)
        tm1 = pp.tile([CR, 1], f32, tag="tm1", name="tm1")
        nc.vector.tensor_mul(tm1, mu1, k1)
        bi1 = pp.tile([CR, 1], f32, tag="bi1", name="bi1")
        nc.vector.tensor_sub(bi1, be1, tm1)

        # BN2 parameters broadcast to [BL, C]
        ga2 = pp.tile([BL, C], f32, tag="ga2", name="ga2")
        nc.gpsimd.dma_start(ga2, bcast(ga2_d))
        be2 = pp.tile([BL, C], f32, tag="be2", name="be2")
        nc.gpsimd.dma_start(be2, bcast(be2_d))
        mu2 = pp.tile([BL, C], f32, tag="mu2", name="mu2")
        nc.gpsimd.dma_start(mu2, bcast(mu2_d))
        va2 = pp.tile([BL, C], f32, tag="va2", name="va2")
        nc.gpsimd.dma_start(va2, bcast(va2_d))

        eps2 = pp.tile([BL, 1], f32, tag="eps2", name="eps2")
        nc.vector.memset(eps2, EPS)
        va2c = pp.tile([BL, C], f32, tag="va2c", name="va2c")
        nc.vector.tensor_copy(va2c, va2)
        srt2 = pp.tile([BL, C], f32, tag="srt2", name="srt2")
        nc.scalar.activation(srt2, va2c, AF.Sqrt, bias=eps2)
        rst2 = pp.tile([BL, C], f32, tag="rst2", name="rst2")
        nc.vector.reciprocal(rst2, srt2)
        k2 = pp.tile([BL, C], f32, tag="k2", name="k2")
        nc.vector.tensor_mul(k2, ga2, rst2)
        tm2 = pp.tile([BL, C], f32, tag="tm2", name="tm2")
        nc.vector.tensor_mul(tm2, mu2, k2)
        bi2 = pp.tile([BL, C], f32, tag="bi2", name="bi2")
        nc.vector.tensor_sub(bi2, be2, tm2)

        # ---- stage 1: squeeze (global sum over H*W per sample/channel) ----
        # acc_sb[p, q, :]: raw [1, 512] channel sums for sample 2q + p/32
        acc_sb = pp.tile([128, 4, 512], f32, tag="acc_sb", name="acc_sb")
        # s_sb[p, q, :]: folded [1, 256] sums
        s_sb = pp.tile([128, 4, C], f32, tag="s_sb", name="s_sb")
        # sT[c, b] per channel half -> MLP rhs
        sT0 = mlpp.tile([128, BL], f32, tag="sT0", name="sT0")
        sT1 = mlpp.tile([128, BL], f32, tag="sT1", name="sT1")

        # Work split inside a sample: of the NK=14 512-wide column slices,
        # PE reduces NPE directly (f32 matmul is 2 HW passes, ~858 ns/slice)
        # and DVE pre-reduces the remaining NDV slices into one [112, 512]
        # partial with a single strided tensor_reduce (~(NDV*512+151)/0.96 ns),
        # which PE then folds in with one extra matmul.
        NDV = 9
        NPE = NK - NDV
        for q in range(4):
            acc = accp.tile([128, 512], f32, tag="acc", name=f"acc{q}")
            for j in range(2):
                b = 2 * q + j
                pb = 32 * j
                xt = xp.tile([P, FD], f32, tag="xt", name=f"xt{b}")
                nc.sync.dma_start(xt, x_d[b])
                for k in range(NPE):
                    nc.tensor.matmul(
                        acc[pb : pb + 1, :],
                        ones_t,
                        xt[:, k * 512 : (k + 1) * 512],
                        start=(k == 0),
                        stop=False,
                    )
                dve_acc = pp.tile([P, 512], f32, tag="dve_acc", name=f"dve_acc{b}", bufs=3)
                nc.vector.tensor_reduce(
                    dve_acc,
                    xt[:, NPE * 512 :].rearrange("p (d f) -> p f d", d=NDV),
                    axis=mybir.AxisListType.X,
                    op=mybir.AluOpType.add,
                )
                nc.tensor.matmul(
                    acc[pb : pb + 1, :],
                    ones_t,
                    dve_acc,
                    start=False,
                    stop=True,
                )
            for j in range(2):
                pb = 32 * j
                nc.scalar.copy(acc_sb[pb : pb + 1, q, :], acc[pb : pb + 1, :])
                nc.vector.tensor_add(
                    s_sb[pb : pb + 1, q, :],
                    acc_sb[pb : pb + 1, q, 0:C],
                    acc_sb[pb : pb + 1, q, C : 2 * C],
                )
            # gather this pair's sums into the sT layout (also the transpose)
            for j in range(2):
                b = 2 * q + j
                pb = 32 * j
                for h, sT in enumerate((sT0, sT1)):
                    nc.tensor.matmul(
                        sT[:, 0:BL],
                        s_sb[pb : pb + 1, q, h * 128 : (h + 1) * 128],
                        oh[pb : pb + 1, b, :],
                        start=(b == 0),
                        stop=(b == BL - 1),
                    )

        # ---- stage 2: excite MLP ----
        sT0s = pp.tile([128, BL], f32, tag="sT0s", name="sT0s")
        nc.scalar.copy(sT0s, sT0)
        sT1s = pp.tile([128, BL], f32, tag="sT1s", name="sT1s")
        nc.vector.tensor_copy(sT1s, sT1)

        g1p = mlpp.tile([CR, BL], f32, tag="g1p", name="g1p")
        nc.tensor.matmul(g1p, w1a, sT0s, start=True, stop=False)
        nc.tensor.matmul(g1p, w1b, sT1s, start=False, stop=True)

        # h = relu(g1 * scale1 + bias1)  (BN1 + mean scale + relu in one op).
        # bi1 comes from DVE; copy it through ACT so the Relu activation's
        # only cross-engine wait is on the PE matmul result.
        bi1c = pp.tile([CR, 1], f32, tag="bi1c", name="bi1c")
        nc.scalar.copy(bi1c, bi1)
        sc1c = pp.tile([CR, 1], f32, tag="sc1c", name="sc1c")
        nc.scalar.copy(sc1c, sc1)
        h_sb = pp.tile([CR, BL], f32, tag="h_sb", name="h_sb")
        nc.scalar.activation(h_sb, g1p, AF.Relu, bias=bi1c, scale=sc1c)

        o_p = mlpp.tile([BL, C], f32, tag="o_p", name="o_p")
        nc.tensor.matmul(o_p, h_sb, w2t, start=True, stop=True)

        osc = pp.tile([BL, C], f32, tag="osc", name="osc")
        nc.vector.tensor_mul(osc, o_p, k2)
        ofin = pp.tile([BL, C], f32, tag="ofin", name="ofin")
        nc.vector.tensor_add(ofin, osc, bi2)
        nc.sync.dma_start(out_d[:, :], ofin)

    nc.compile()
    return nc


def _get_nc():
    if "nc" not in _CACHE:
        _CACHE["nc"] = _build_nc()
    return _CACHE["nc"]


def _in_maps(inputs):
    x = np.ascontiguousarray(np.asarray(inputs["x"], dtype=np.float32))
    w1 = np.ascontiguousarray(np.asarray(inputs["w1"], dtype=np.float32))
    w2 = np.ascontiguousarray(np.asarray(inputs["w2"], dtype=np.float32))
    p1 = {
        k: np.ascontiguousarray(
            np.asarray(inputs[k], dtype=np.float32).reshape(CR, 1)
        )
        for k in ("gamma1", "beta1", "mean1", "var1")
    }
    p2 = {
        k: np.ascontiguousarray(np.asarray(inputs[k], dtype=np.float32).reshape(C))
        for k in ("gamma2", "beta2", "mean2", "var2")
    }
    maps = []
    for c in range(NCORES):
        shard = np.ascontiguousarray(x[c * BL : (c + 1) * BL]).reshape(BL, P, FD)
        maps.append({"x": shard, "w1": w1, "w2": w2, **p1, **p2})
    return maps


def _run(inputs, trace=False):
    from concourse.bass_utils import run_bass_kernel_spmd

    nc = _get_nc()
    res = run_bass_kernel_spmd(
        nc, _in_maps(inputs), core_ids=list(range(NCORES)), trace=trace
    )
    out = np.concatenate([res.results[c]["out"] for c in range(NCORES)], axis=0)
    return out.reshape(B, 1, 1, C).astype(np.float32), res


def kernel(**inputs) -> np.ndarray:
    out, _ = _run(inputs, trace=False)
    return out


def kernel_traced(**inputs):
    """Returns (out, BassKernelResults) with NTFF profiling enabled."""
    return _run(inputs, trace=True)


def bench(inputs, iters=30, warmup=5):
    """Time the per-step NEFF execution with device-resident inputs.

    Returns (out_full, per_call_seconds_list). Inputs are device_put once;
    each timed call only dispatches the compiled executable, so steady-state
    per-call wall time ~= max-core NEFF exec + dispatch overhead.
    """
    import time
    import jax
    import jax.numpy as jnp
    from jax.sharding import Mesh, PartitionSpec, NamedSharding
    from jax.experimental.shard_map import shard_map
    from concourse import bass2jax, mybir

    bass2jax.install_neuronx_cc_hook()
    nc = _get_nc()

    partition_name = nc.partition_id_tensor.name if nc.partition_id_tensor else None
    in_names, out_names, out_avals = [], [], []
    for alloc in nc.m.functions[0].allocations:
        if not isinstance(alloc, mybir.MemoryLocationSet):
            continue
        name = alloc.memorylocations[0].name
        if alloc.kind == "ExternalInput":
            if name != partition_name:
                in_names.append(name)
        elif alloc.kind == "ExternalOutput":
            out_names.append(name)
            out_avals.append(
                jax.core.ShapedArray(tuple(alloc.tensor_shape), mybir.dt.np(alloc.dtype))
            )
    all_in_names = in_names + out_names
    if partition_name is not None:
        all_in_names = all_in_names + [partition_name]

    def _body(*operands):
        operands = list(operands)
        if partition_name is not None:
            operands.append(bass2jax.partition_id_tensor())
        outs = bass2jax._bass_exec_p.bind(
            *operands,
            out_avals=tuple(out_avals),
            in_names=tuple(all_in_names),
            out_names=tuple(out_names),
            lowering_input_output_aliases=(),
            sim_require_finite=True,
            sim_require_nnan=True,
            nc=nc,
        )
        return tuple(outs)

    devices = jax.devices()[:NCORES]
    mesh = Mesh(np.asarray(devices), ("core",))
    spec = PartitionSpec("core")
    maps = _in_maps(inputs)
    concat = [
        np.concatenate([maps[c][n] for c in range(NCORES)], axis=0) for n in in_names
    ]
    concat += [
        np.zeros((NCORES * a.shape[0], *a.shape[1:]), a.dtype) for a in out_avals
    ]
    sharding = NamedSharding(mesh, spec)
    dev_in = [jax.device_put(a, sharding) for a in concat]

    fn = jax.jit(
        shard_map(
            _body,
            mesh=mesh,
            in_specs=(spec,) * len(concat),
            out_specs=(spec,) * len(out_names),
            check_rep=False,
        )
    )

    for _ in range(warmup):
        outs = fn(*dev_in)
    jax.block_until_ready(outs)

    times = []
    for _ in range(iters):
        t0 = time.perf_counter()
        outs = fn(*dev_in)
        jax.block_until_ready(outs)
        times.append(time.perf_counter() - t0)

    oidx = out_names.index("out")
    o = np.asarray(outs[oidx]).reshape(NCORES, BL, C).reshape(B, C)
    return o.reshape(B, 1, 1, C).astype(np.float32), times


# revision 23
# speedup vs baseline: 1.2507x; 1.2040x over previous
"""Trainium2 Bass kernel for nn_ChannelAttention (squeeze-excite).

Reference computation:
    s = mean(x, axis=(H, W))                    # [B, C]   global avg pool
    h = relu(bn1(s @ w1))                       # [B, Cr]  Cr = 16
    o = bn2(h @ w2)                             # [B, C]
    return o[:, None, None, :]                  # [B, 1, 1, C]

Strategy (data-parallel over batch, 8 cores x 8 samples):
  - Each core streams its 8 samples (3.2 MB each, one HWDGE DMA per sample)
    into SBUF tiles of [112 partitions, 7168] (28 spatial rows per partition,
    channel-aligned since 28*256 = 7168).
  - Squeeze: partition-axis reduction via TensorE with a ones[112,1] lhsT,
    accumulating per-sample channel sums into PSUM ([1,512] per sample; the
    512 free dim holds channel c in both col c and c+256, folded later).
  - Per-sample sums are copied to SBUF, folded to [1,256], then gathered into
    an sT layout [128ch, 8samples] x2 via K=1 one-hot matmuls (this doubles
    as the transpose needed for the excite MLP).
  - Excite MLP on PE: g1[16,8] = w1.T @ sT (K=256 split in 2), BN1+ReLU as a
    single ScalarE activation (per-partition scale/bias APs, with the 1/HW
    mean scale folded into BN1's scale), o[8,256] = h.T @ w2, BN2 applied
    with parameters broadcast to [8,256] via zero-stride DMA.
"""

import sys

if "/opt/trn_rl_repo" not in sys.path:
    sys.path.insert(0, "/opt/trn_rl_repo")

import numpy as np

B, H, W, C = 64, 56, 56, 256
CR = 16
NCORES = 8
BL = B // NCORES  # samples per core
HWP = H * W  # 3136 spatial positions
NPAIR = BL // 2  # 4 sample-pairs per core, one DMA each
PFD = 2 * HWP * C // 128  # 12544 free-dim elements per partition (49 rows)
NSL = PFD // 512  # 24 full 512-wide column slices (+ one 256 tail)
EPS = 1e-3

_CACHE: dict = {}


def _build_nc():
    import concourse.bass as bass
    import concourse.tile as tile
    from concourse import bacc, mybir
    from contextlib import ExitStack

    f32 = mybir.dt.float32
    AF = mybir.ActivationFunctionType

    nc = bacc.Bacc("TRN2", target_bir_lowering=False, debug=False)

    x_d = nc.dram_tensor("x", [NPAIR, 128, PFD], f32, kind="ExternalInput")
    w1_d = nc.dram_tensor("w1", [C, CR], f32, kind="ExternalInput")
    ga1_d = nc.dram_tensor("gamma1", [CR, 1], f32, kind="ExternalInput")
    be1_d = nc.dram_tensor("beta1", [CR, 1], f32, kind="ExternalInput")
    mu1_d = nc.dram_tensor("mean1", [CR, 1], f32, kind="ExternalInput")
    va1_d = nc.dram_tensor("var1", [CR, 1], f32, kind="ExternalInput")
    w2_d = nc.dram_tensor("w2", [CR, C], f32, kind="ExternalInput")
    ga2_d = nc.dram_tensor("gamma2", [C], f32, kind="ExternalInput")
    be2_d = nc.dram_tensor("beta2", [C], f32, kind="ExternalInput")
    mu2_d = nc.dram_tensor("mean2", [C], f32, kind="ExternalInput")
    va2_d = nc.dram_tensor("var2", [C], f32, kind="ExternalInput")
    out_d = nc.dram_tensor("out", [BL, C], f32, kind="ExternalOutput")

    def bcast(d):
        # [C] dram vector -> [BL, C] read AP with zero partition stride
        a = d[:]
        return bass.AP(tensor=a.tensor, offset=a.offset, ap=[[0, BL], [1, C]])

    with ExitStack() as ctx:
        tc = ctx.enter_context(tile.TileContext(nc))
        xp = ctx.enter_context(tc.tile_pool(name="xp", bufs=4))
        pp = ctx.enter_context(tc.tile_pool(name="pp", bufs=1))
        accp = ctx.enter_context(tc.tile_pool(name="accp", bufs=4, space="PSUM"))
        mlpp = ctx.enter_context(tc.tile_pool(name="mlpp", bufs=1, space="PSUM"))

        # ---- constants / parameters (all overlap with the main stream) ----
        # pair indicator, M=33 so the two samples' sums land on the
        # 32-aligned PSUM partitions {0, 32} (compute-engine APs require
        # 32-aligned partition bases): col 0 selects partitions 0..63
        # (first sample of the pair), col 32 selects 64..127 (second).
        po = pp.tile([128, 33], f32, tag="po", name="po")
        nc.vector.memset(po, 0.0)
        nc.vector.memset(po[0:64, 0:1], 1.0)
        nc.vector.memset(po[64:128, 32:33], 1.0)

        # one-hot bank: oh[p, b, j] = (b == j), identical on every partition
        oh = pp.tile([128, BL, BL], f32, tag="oh", name="oh")
        nc.vector.memset(oh, 0.0)
        for b in range(BL):
            nc.vector.memset(oh[:, b, b : b + 1], 1.0)

        w1a = pp.tile([128, CR], f32, tag="w1a", name="w1a")
        nc.sync.dma_start(w1a, w1_d[0:128, :])
        w1b = pp.tile([128, CR], f32, tag="w1b", name="w1b")
        nc.sync.dma_start(w1b, w1_d[128:256, :])
        w2t = pp.tile([CR, C], f32, tag="w2t", name="w2t")
        nc.sync.dma_start(w2t, w2_d[:, :])

        # BN1 parameters, [16, 1] per-partition layout
        ga1 = pp.tile([CR, 1], f32, tag="ga1", name="ga1")
        nc.sync.dma_start(ga1, ga1_d[:, :])
        be1 = pp.tile([CR, 1], f32, tag="be1", name="be1")
        nc.sync.dma_start(be1, be1_d[:, :])
        mu1 = pp.tile([CR, 1], f32, tag="mu1", name="mu1")
        nc.sync.dma_start(mu1, mu1_d[:, :])
        va1 = pp.tile([CR, 1], f32, tag="va1", name="va1")
        nc.sync.dma_start(va1, va1_d[:, :])

        # scale1 = gamma1 / sqrt(var1 + eps) / HW, bias1 = beta1 - mean1 * k1
        # (route activation deps through a single engine: the Activation
        # instruction encoding only has room for one sync wait when bias
        # is an AP, so both of its inputs must come from the same sem)
        eps1 = pp.tile([CR, 1], f32, tag="eps1", name="eps1")
        nc.vector.memset(eps1, EPS)
        va1c = pp.tile([CR, 1], f32, tag="va1c", name="va1c")
        nc.vector.tensor_copy(va1c, va1)
        srt1 = pp.tile([CR, 1], f32, tag="srt1", name="srt1")
        nc.scalar.activation(srt1, va1c, AF.Sqrt, bias=eps1)
        rst1 = pp.tile([CR, 1], f32, tag="rst1", name="rst1")
        nc.vector.reciprocal(rst1, srt1)
        k1 = pp.tile([CR, 1], f32, tag="k1", name="k1")
        nc.vector.tensor_mul(k1, ga1, rst1)
        sc1 = pp.tile([CR, 1], f32, tag="sc1", name="sc1")
        nc.scalar.mul(sc1, k1, 1.0 / HWP)
        tm1 = pp.tile([CR, 1], f32, tag="tm1", name="tm1")
        nc.vector.tensor_mul(tm1, mu1, k1)
        bi1 = pp.tile([CR, 1], f32, tag="bi1", name="bi1")
        nc.vector.tensor_sub(bi1, be1, tm1)

        # BN2 parameters broadcast to [BL, C]
        ga2 = pp.tile([BL, C], f32, tag="ga2", name="ga2")
        nc.gpsimd.dma_start(ga2, bcast(ga2_d))
        be2 = pp.tile([BL, C], f32, tag="be2", name="be2")
        nc.gpsimd.dma_start(be2, bcast(be2_d))
        mu2 = pp.tile([BL, C], f32, tag="mu2", name="mu2")
        nc.gpsimd.dma_start(mu2, bcast(mu2_d))
        va2 = pp.tile([BL, C], f32, tag="va2", name="va2")
        nc.gpsimd.dma_start(va2, bcast(va2_d))

        eps2 = pp.tile([BL, 1], f32, tag="eps2", name="eps2")
        nc.vector.memset(eps2, EPS)
        va2c = pp.tile([BL, C], f32, tag="va2c", name="va2c")
        nc.vector.tensor_copy(va2c, va2)
        srt2 = pp.tile([BL, C], f32, tag="srt2", name="srt2")
        nc.scalar.activation(srt2, va2c, AF.Sqrt, bias=eps2)
        rst2 = pp.tile([BL, C], f32, tag="rst2", name="rst2")
        nc.vector.reciprocal(rst2, srt2)
        k2 = pp.tile([BL, C], f32, tag="k2", name="k2")
        nc.vector.tensor_mul(k2, ga2, rst2)
        tm2 = pp.tile([BL, C], f32, tag="tm2", name="tm2")
        nc.vector.tensor_mul(tm2, mu2, k2)
        bi2 = pp.tile([BL, C], f32, tag="bi2", name="bi2")
        nc.vector.tensor_sub(bi2, be2, tm2)

        # ---- stage 1: squeeze (global sum over H*W per sample/channel) ----
        # acc_sb[32j, q, :]: raw [1, 512] channel sums for sample 2q + j
        acc_sb = pp.tile([128, NPAIR, 512], f32, tag="acc_sb", name="acc_sb")
        # s_sb[32j, q, :]: folded [1, 256] sums
        s_sb = pp.tile([128, NPAIR, C], f32, tag="s_sb", name="s_sb")
        # sT[c, b] per channel half -> MLP rhs
        sT0 = mlpp.tile([128, BL], f32, tag="sT0", name="sT0")
        sT1 = mlpp.tile([128, BL], f32, tag="sT1", name="sT1")

        # Work split inside a pair tile: of the 24 full 512-wide column
        # slices, PE reduces NPE directly with the [128,2] pair-indicator
        # lhsT (f32 matmul = 2 HW passes, ~858 ns/slice) plus the 256-wide
        # tail; DVE pre-reduces the remaining NDV slices with a chain of
        # tensor_adds (~690 ns each), folded in by one extra PE matmul.
        NPE = 6
        NDV = NSL - NPE
        for q in range(NPAIR):
            xt = xp.tile([128, PFD], f32, tag="xt", name=f"xt{q}", bufs=3)
            nc.sync.dma_start(xt, x_d[q])
            acc = accp.tile([128, 512], f32, tag="acc", name=f"acc{q}")
            for k in range(NPE):
                nc.tensor.matmul(
                    acc[0:33, :],
                    po,
                    xt[:, k * 512 : (k + 1) * 512],
                    start=(k == 0),
                    stop=False,
                )
            # 256-wide tail column slice
            nc.tensor.matmul(
                acc[0:33, 0:256],
                po,
                xt[:, NSL * 512 :],
                start=False,
                stop=False,
            )
            dve_acc = pp.tile([128, 512], f32, tag="dve_acc", name=f"dve{q}", bufs=3)
            nc.vector.tensor_add(
                dve_acc,
                xt[:, NPE * 512 : (NPE + 1) * 512],
                xt[:, (NPE + 1) * 512 : (NPE + 2) * 512],
            )
            for k in range(NPE + 2, NSL):
                nc.vector.tensor_add(
                    dve_acc, dve_acc, xt[:, k * 512 : (k + 1) * 512]
                )
            nc.tensor.matmul(acc[0:33, :], po, dve_acc, start=False, stop=True)

            for j in range(2):
                pb = 32 * j
                nc.scalar.copy(acc_sb[pb : pb + 1, q, :], acc[pb : pb + 1, :])
                nc.vector.tensor_add(
                    s_sb[pb : pb + 1, q, :],
                    acc_sb[pb : pb + 1, q, 0:C],
                    acc_sb[pb : pb + 1, q, C : 2 * C],
                )
            # gather this pair's sums into the sT layout (also the transpose)
            for j in range(2):
                b = 2 * q + j
                pb = 32 * j
                for h, sT in enumerate((sT0, sT1)):
                    nc.tensor.matmul(
                        sT[:, 0:BL],
                        s_sb[pb : pb + 1, q, h * 128 : (h + 1) * 128],
                        oh[pb : pb + 1, b, :],
                        start=(b == 0),
                        stop=(b == BL - 1),
                    )

        # ---- stage 2: excite MLP ----
        sT0s = pp.tile([128, BL], f32, tag="sT0s", name="sT0s")
        nc.scalar.copy(sT0s, sT0)
        sT1s = pp.tile([128, BL], f32, tag="sT1s", name="sT1s")
        nc.vector.tensor_copy(sT1s, sT1)

        g1p = mlpp.tile([CR, BL], f32, tag="g1p", name="g1p")
        nc.tensor.matmul(g1p, w1a, sT0s, start=True, stop=False)
        nc.tensor.matmul(g1p, w1b, sT1s, start=False, stop=True)

        # h = relu(g1 * scale1 + bias1)  (BN1 + mean scale + relu in one op).
        # bi1 comes from DVE; copy it through ACT so the Relu activation's
        # only cross-engine wait is on the PE matmul result.
        bi1c = pp.tile([CR, 1], f32, tag="bi1c", name="bi1c")
        nc.scalar.copy(bi1c, bi1)
        sc1c = pp.tile([CR, 1], f32, tag="sc1c", name="sc1c")
        nc.scalar.copy(sc1c, sc1)
        h_sb = pp.tile([CR, BL], f32, tag="h_sb", name="h_sb")
        nc.scalar.activation(h_sb, g1p, AF.Relu, bias=bi1c, scale=sc1c)

        o_p = mlpp.tile([BL, C], f32, tag="o_p", name="o_p")
        nc.tensor.matmul(o_p, h_sb, w2t, start=True, stop=True)

        osc = pp.tile([BL, C], f32, tag="osc", name="osc")
        nc.vector.tensor_mul(osc, o_p, k2)
        ofin = pp.tile([BL, C], f32, tag="ofin", name="ofin")
        nc.vector.tensor_add(ofin, osc, bi2)
        nc.sync.dma_start(out_d[:, :], ofin)

    nc.compile()
    return nc


def _get_nc():
    if "nc" not in _CACHE:
        _CACHE["nc"] = _build_nc()
    return _CACHE["nc"]


def _in_maps(inputs):
    x = np.ascontiguousarray(np.asarray(inputs["x"], dtype=np.float32))
    w1 = np.ascontiguousarray(np.asarray(inputs["w1"], dtype=np.float32))
    w2 = np.ascontiguousarray(np.asarray(inputs["w2"], dtype=np.float32))
    p1 = {
        k: np.ascontiguousarray(
            np.asarray(inputs[k], dtype=np.float32).reshape(CR, 1)
        )
        for k in ("gamma1", "beta1", "mean1", "var1")
    }
    p2 = {
        k: np.ascontiguousarray(np.asarray(inputs[k], dtype=np.float32).reshape(C))
        for k in ("gamma2", "beta2", "mean2", "var2")
    }
    maps = []
    for c in range(NCORES):
        shard = np.ascontiguousarray(x[c * BL : (c + 1) * BL]).reshape(NPAIR, 128, PFD)
        maps.append({"x": shard, "w1": w1, "w2": w2, **p1, **p2})
    return maps


def _run(inputs, trace=False):
    from concourse.bass_utils import run_bass_kernel_spmd

    nc = _get_nc()
    res = run_bass_kernel_spmd(
        nc, _in_maps(inputs), core_ids=list(range(NCORES)), trace=trace
    )
    out = np.concatenate([res.results[c]["out"] for c in range(NCORES)], axis=0)
    return out.reshape(B, 1, 1, C).astype(np.float32), res


def kernel(**inputs) -> np.ndarray:
    out, _ = _run(inputs, trace=False)
    return out


def kernel_traced(**inputs):
    """Returns (out, BassKernelResults) with NTFF profiling enabled."""
    return _run(inputs, trace=True)


def bench(inputs, iters=30, warmup=5):
    """Time the per-step NEFF execution with device-resident inputs.

    Returns (out_full, per_call_seconds_list). Inputs are device_put once;
    each timed call only dispatches the compiled executable, so steady-state
    per-call wall time ~= max-core NEFF exec + dispatch overhead.
    """
    import time
    import jax
    import jax.numpy as jnp
    from jax.sharding import Mesh, PartitionSpec, NamedSharding
    from jax.experimental.shard_map import shard_map
    from concourse import bass2jax, mybir

    bass2jax.install_neuronx_cc_hook()
    nc = _get_nc()

    partition_name = nc.partition_id_tensor.name if nc.partition_id_tensor else None
    in_names, out_names, out_avals = [], [], []
    for alloc in nc.m.functions[0].allocations:
        if not isinstance(alloc, mybir.MemoryLocationSet):
            continue
        name = alloc.memorylocations[0].name
        if alloc.kind == "ExternalInput":
            if name != partition_name:
                in_names.append(name)
        elif alloc.kind == "ExternalOutput":
            out_names.append(name)
            out_avals.append(
                jax.core.ShapedArray(tuple(alloc.tensor_shape), mybir.dt.np(alloc.dtype))
            )
    all_in_names = in_names + out_names
    if partition_name is not None:
        all_in_names = all_in_names + [partition_name]

    def _body(*operands):
        operands = list(operands)
        if partition_name is not None:
            operands.append(bass2jax.partition_id_tensor())
        outs = bass2jax._bass_exec_p.bind(
            *operands,
            out_avals=tuple(out_avals),
            in_names=tuple(all_in_names),
            out_names=tuple(out_names),
            lowering_input_output_aliases=(),
            sim_require_finite=True,
            sim_require_nnan=True,
            nc=nc,
        )
        return tuple(outs)

    devices = jax.devices()[:NCORES]
    mesh = Mesh(np.asarray(devices), ("core",))
    spec = PartitionSpec("core")
    maps = _in_maps(inputs)
    concat = [
        np.concatenate([maps[c][n] for c in range(NCORES)], axis=0) for n in in_names
    ]
    concat += [
        np.zeros((NCORES * a.shape[0], *a.shape[1:]), a.dtype) for a in out_avals
    ]
    sharding = NamedSharding(mesh, spec)
    dev_in = [jax.device_put(a, sharding) for a in concat]

    fn = jax.jit(
        shard_map(
            _body,
            mesh=mesh,
            in_specs=(spec,) * len(concat),
            out_specs=(spec,) * len(out_names),
            check_rep=False,
        )
    )

    for _ in range(warmup):
        outs = fn(*dev_in)
    jax.block_until_ready(outs)

    times = []
    for _ in range(iters):
        t0 = time.perf_counter()
        outs = fn(*dev_in)
        jax.block_until_ready(outs)
        times.append(time.perf_counter() - t0)

    oidx = out_names.index("out")
    o = np.asarray(outs[oidx]).reshape(NCORES, BL, C).reshape(B, C)
    return o.reshape(B, 1, 1, C).astype(np.float32), times


# revision 26
# speedup vs baseline: 1.4953x; 1.1956x over previous
"""Trainium2 Bass kernel for nn_ChannelAttention (squeeze-excite).

Reference computation:
    s = mean(x, axis=(H, W))                    # [B, C]   global avg pool
    h = relu(bn1(s @ w1))                       # [B, Cr]  Cr = 16
    o = bn2(h @ w2)                             # [B, C]
    return o[:, None, None, :]                  # [B, 1, 1, C]

Strategy (data-parallel over batch, 8 cores x 8 samples):
  - Each core streams its 8 samples (3.2 MB each, one HWDGE DMA per sample)
    into SBUF tiles of [112 partitions, 7168] (28 spatial rows per partition,
    channel-aligned since 28*256 = 7168).
  - Squeeze: partition-axis reduction via TensorE with a ones[112,1] lhsT,
    accumulating per-sample channel sums into PSUM ([1,512] per sample; the
    512 free dim holds channel c in both col c and c+256, folded later).
  - Per-sample sums are copied to SBUF, folded to [1,256], then gathered into
    an sT layout [128ch, 8samples] x2 via K=1 one-hot matmuls (this doubles
    as the transpose needed for the excite MLP).
  - Excite MLP on PE: g1[16,8] = w1.T @ sT (K=256 split in 2), BN1+ReLU as a
    single ScalarE activation (per-partition scale/bias APs, with the 1/HW
    mean scale folded into BN1's scale), o[8,256] = h.T @ w2, BN2 applied
    with parameters broadcast to [8,256] via zero-stride DMA.
"""

import sys

if "/opt/trn_rl_repo" not in sys.path:
    sys.path.insert(0, "/opt/trn_rl_repo")

import numpy as np

B, H, W, C = 64, 56, 56, 256
CR = 16
NCORES = 8
BL = B // NCORES  # samples per core
HWP = H * W  # 3136 spatial positions
NPAIR = BL // 2  # 4 sample-pairs per core, one DMA each
PFD = 2 * HWP * C // 128  # 12544 free-dim elements per partition (49 rows)
NSL = PFD // 512  # 24 full 512-wide column slices (+ one 256 tail)
EPS = 1e-3

_CACHE: dict = {}


def _build_nc():
    import concourse.bass as bass
    import concourse.tile as tile
    from concourse import bacc, mybir
    from contextlib import ExitStack

    f32 = mybir.dt.float32
    AF = mybir.ActivationFunctionType

    nc = bacc.Bacc("TRN2", target_bir_lowering=False, debug=False)

    x_d = nc.dram_tensor("x", [NPAIR, 128, PFD], f32, kind="ExternalInput")
    w1_d = nc.dram_tensor("w1", [C, CR], f32, kind="ExternalInput")
    ga1_d = nc.dram_tensor("gamma1", [CR, 1], f32, kind="ExternalInput")
    be1_d = nc.dram_tensor("beta1", [CR, 1], f32, kind="ExternalInput")
    mu1_d = nc.dram_tensor("mean1", [CR, 1], f32, kind="ExternalInput")
    va1_d = nc.dram_tensor("var1", [CR, 1], f32, kind="ExternalInput")
    w2_d = nc.dram_tensor("w2", [CR, C], f32, kind="ExternalInput")
    ga2_d = nc.dram_tensor("gamma2", [C], f32, kind="ExternalInput")
    be2_d = nc.dram_tensor("beta2", [C], f32, kind="ExternalInput")
    mu2_d = nc.dram_tensor("mean2", [C], f32, kind="ExternalInput")
    va2_d = nc.dram_tensor("var2", [C], f32, kind="ExternalInput")
    out_d = nc.dram_tensor("out", [BL, C], f32, kind="ExternalOutput")

    def bcast(d):
        # [C] dram vector -> [BL, C] read AP with zero partition stride
        a = d[:]
        return bass.AP(tensor=a.tensor, offset=a.offset, ap=[[0, BL], [1, C]])

    with ExitStack() as ctx:
        tc = ctx.enter_context(tile.TileContext(nc))
        xp = ctx.enter_context(tc.tile_pool(name="xp", bufs=4))
        pp = ctx.enter_context(tc.tile_pool(name="pp", bufs=1))
        accp = ctx.enter_context(tc.tile_pool(name="accp", bufs=4, space="PSUM"))
        mlpp = ctx.enter_context(tc.tile_pool(name="mlpp", bufs=1, space="PSUM"))

        # ---- x stream first: the sync HWDGE ring runs FIFO, so the big
        # DMAs are issued before anything else queues on it. The last pair
        # is split in two halves so its consumption can start ~8 us before
        # the final bytes land (shorter kernel tail).
        xts = []
        for q in range(NPAIR):
            xt = xp.tile([128, PFD], f32, tag="xt", name=f"xt{q}", bufs=3)
            if q < NPAIR - 1:
                nc.sync.dma_start(xt, x_d[q])
            else:
                nc.sync.dma_start(xt[:, 0:6144], x_d[q][:, 0:6144])
                nc.sync.dma_start(xt[:, 6144:PFD], x_d[q][:, 6144:PFD])
            xts.append(xt)

        # ---- constants / parameters (all overlap with the main stream) ----
        # pair indicator, M=33 so the two samples' sums land on the
        # 32-aligned PSUM partitions {0, 32} (compute-engine APs require
        # 32-aligned partition bases): col 0 selects partitions 0..63
        # (first sample of the pair), col 32 selects 64..127 (second).
        po = pp.tile([128, 33], f32, tag="po", name="po")
        nc.vector.memset(po, 0.0)
        nc.vector.memset(po[0:64, 0:1], 1.0)
        nc.vector.memset(po[64:128, 32:33], 1.0)

        # one-hot bank: oh[p, b, j] = (b == j), identical on every partition
        oh = pp.tile([128, BL, BL], f32, tag="oh", name="oh")
        nc.vector.memset(oh, 0.0)
        for b in range(BL):
            nc.vector.memset(oh[:, b, b : b + 1], 1.0)

        w1a = pp.tile([128, CR], f32, tag="w1a", name="w1a")
        nc.scalar.dma_start(w1a, w1_d[0:128, :])
        w1b = pp.tile([128, CR], f32, tag="w1b", name="w1b")
        nc.scalar.dma_start(w1b, w1_d[128:256, :])
        w2t = pp.tile([CR, C], f32, tag="w2t", name="w2t")
        nc.scalar.dma_start(w2t, w2_d[:, :])

        # BN1 parameters, [16, 1] per-partition layout
        ga1 = pp.tile([CR, 1], f32, tag="ga1", name="ga1")
        nc.scalar.dma_start(ga1, ga1_d[:, :])
        be1 = pp.tile([CR, 1], f32, tag="be1", name="be1")
        nc.scalar.dma_start(be1, be1_d[:, :])
        mu1 = pp.tile([CR, 1], f32, tag="mu1", name="mu1")
        nc.scalar.dma_start(mu1, mu1_d[:, :])
        va1 = pp.tile([CR, 1], f32, tag="va1", name="va1")
        nc.scalar.dma_start(va1, va1_d[:, :])

        # scale1 = gamma1 / sqrt(var1 + eps) / HW, bias1 = beta1 - mean1 * k1
        # (route activation deps through a single engine: the Activation
        # instruction encoding only has room for one sync wait when bias
        # is an AP, so both of its inputs must come from the same sem)
        eps1 = pp.tile([CR, 1], f32, tag="eps1", name="eps1")
        nc.vector.memset(eps1, EPS)
        va1c = pp.tile([CR, 1], f32, tag="va1c", name="va1c")
        nc.vector.tensor_copy(va1c, va1)
        srt1 = pp.tile([CR, 1], f32, tag="srt1", name="srt1")
        nc.scalar.activation(srt1, va1c, AF.Sqrt, bias=eps1)
        rst1 = pp.tile([CR, 1], f32, tag="rst1", name="rst1")
        nc.vector.reciprocal(rst1, srt1)
        k1 = pp.tile([CR, 1], f32, tag="k1", name="k1")
        nc.vector.tensor_mul(k1, ga1, rst1)
        sc1 = pp.tile([CR, 1], f32, tag="sc1", name="sc1")
        nc.scalar.mul(sc1, k1, 1.0 / HWP)
        tm1 = pp.tile([CR, 1], f32, tag="tm1", name="tm1")
        nc.vector.tensor_mul(tm1, mu1, k1)
        bi1 = pp.tile([CR, 1], f32, tag="bi1", name="bi1")
        nc.vector.tensor_sub(bi1, be1, tm1)

        # BN2 parameters broadcast to [BL, C]
        ga2 = pp.tile([BL, C], f32, tag="ga2", name="ga2")
        nc.gpsimd.dma_start(ga2, bcast(ga2_d))
        be2 = pp.tile([BL, C], f32, tag="be2", name="be2")
        nc.gpsimd.dma_start(be2, bcast(be2_d))
        mu2 = pp.tile([BL, C], f32, tag="mu2", name="mu2")
        nc.gpsimd.dma_start(mu2, bcast(mu2_d))
        va2 = pp.tile([BL, C], f32, tag="va2", name="va2")
        nc.gpsimd.dma_start(va2, bcast(va2_d))

        eps2 = pp.tile([BL, 1], f32, tag="eps2", name="eps2")
        nc.vector.memset(eps2, EPS)
        va2c = pp.tile([BL, C], f32, tag="va2c", name="va2c")
        nc.vector.tensor_copy(va2c, va2)
        srt2 = pp.tile([BL, C], f32, tag="srt2", name="srt2")
        nc.scalar.activation(srt2, va2c, AF.Sqrt, bias=eps2)
        rst2 = pp.tile([BL, C], f32, tag="rst2", name="rst2")
        nc.vector.reciprocal(rst2, srt2)
        k2 = pp.tile([BL, C], f32, tag="k2", name="k2")
        nc.vector.tensor_mul(k2, ga2, rst2)
        tm2 = pp.tile([BL, C], f32, tag="tm2", name="tm2")
        nc.vector.tensor_mul(tm2, mu2, k2)
        bi2 = pp.tile([BL, C], f32, tag="bi2", name="bi2")
        nc.vector.tensor_sub(bi2, be2, tm2)

        # ---- stage 1: squeeze (global sum over H*W per sample/channel) ----
        # acc_sb[32j, q, :]: raw [1, 512] channel sums for sample 2q + j
        acc_sb = pp.tile([128, NPAIR, 512], f32, tag="acc_sb", name="acc_sb")
        # s_sb[32j, q, :]: folded [1, 256] sums
        s_sb = pp.tile([128, NPAIR, C], f32, tag="s_sb", name="s_sb")
        # sT[c, b] per channel half -> MLP rhs
        sT0 = mlpp.tile([128, BL], f32, tag="sT0", name="sT0")
        sT1 = mlpp.tile([128, BL], f32, tag="sT1", name="sT1")

        # Work split inside a pair tile: of the 24 full 512-wide column
        # slices, PE reduces some directly with the pair-indicator lhsT
        # (f32 matmul = 2 HW passes, ~858 ns/slice) plus the 256-wide
        # tail; DVE pre-reduces the rest with chains of tensor_adds
        # (~726 ns each), each chain folded in by one extra PE matmul.
        # Per-pair segment config: list of (pe_slices, dve_slices) so the
        # split last pair can consume each half-DMA independently.
        segs = {q: [(range(0, 8), range(8, NSL))] for q in range(NPAIR - 1)}
        segs[NPAIR - 1] = [
            (range(0, 3), range(3, 12)),
            (range(12, 15), range(15, NSL)),
        ]
        ndve = 0
        for q in range(NPAIR):
            xt = xts[q]
            acc = accp.tile([128, 512], f32, tag="acc", name=f"acc{q}")
            first = True
            for si, (pe_ks, dve_ks) in enumerate(segs[q]):
                last_seg = si == len(segs[q]) - 1
                for k in pe_ks:
                    nc.tensor.matmul(
                        acc[0:33, :],
                        po,
                        xt[:, k * 512 : (k + 1) * 512],
                        start=first,
                        stop=False,
                    )
                    first = False
                if last_seg:
                    # 256-wide tail column slice
                    nc.tensor.matmul(
                        acc[0:33, 0:256],
                        po,
                        xt[:, NSL * 512 :],
                        start=False,
                        stop=False,
                    )
                dve_ks = list(dve_ks)
                dve_acc = pp.tile(
                    [128, 512], f32, tag="dve_acc", name=f"dve{ndve}", bufs=3
                )
                ndve += 1
                nc.vector.tensor_add(
                    dve_acc,
                    xt[:, dve_ks[0] * 512 : (dve_ks[0] + 1) * 512],
                    xt[:, dve_ks[1] * 512 : (dve_ks[1] + 1) * 512],
                )
                for k in dve_ks[2:]:
                    nc.vector.tensor_add(
                        dve_acc, dve_acc, xt[:, k * 512 : (k + 1) * 512]
                    )
                nc.tensor.matmul(
                    acc[0:33, :], po, dve_acc, start=False, stop=last_seg
                )

            for j in range(2):
                pb = 32 * j
                nc.scalar.copy(acc_sb[pb : pb + 1, q, :], acc[pb : pb + 1, :])
                nc.vector.tensor_add(
                    s_sb[pb : pb + 1, q, :],
                    acc_sb[pb : pb + 1, q, 0:C],
                    acc_sb[pb : pb + 1, q, C : 2 * C],
                )
            # gather this pair's sums into the sT layout (also the transpose)
            for j in range(2):
                b = 2 * q + j
                pb = 32 * j
                for h, sT in enumerate((sT0, sT1)):
                    nc.tensor.matmul(
                        sT[:, 0:BL],
                        s_sb[pb : pb + 1, q, h * 128 : (h + 1) * 128],
                        oh[pb : pb + 1, b, :],
                        start=(b == 0),
                        stop=(b == BL - 1),
                    )

        # ---- stage 2: excite MLP ----
        sT0s = pp.tile([128, BL], f32, tag="sT0s", name="sT0s")
        nc.scalar.copy(sT0s, sT0)
        sT1s = pp.tile([128, BL], f32, tag="sT1s", name="sT1s")
        nc.vector.tensor_copy(sT1s, sT1)

        g1p = mlpp.tile([CR, BL], f32, tag="g1p", name="g1p")
        nc.tensor.matmul(g1p, w1a, sT0s, start=True, stop=False)
        nc.tensor.matmul(g1p, w1b, sT1s, start=False, stop=True)

        # h = relu(g1 * scale1 + bias1)  (BN1 + mean scale + relu in one op).
        # bi1 comes from DVE; copy it through ACT so the Relu activation's
        # only cross-engine wait is on the PE matmul result.
        bi1c = pp.tile([CR, 1], f32, tag="bi1c", name="bi1c")
        nc.scalar.copy(bi1c, bi1)
        sc1c = pp.tile([CR, 1], f32, tag="sc1c", name="sc1c")
        nc.scalar.copy(sc1c, sc1)
        h_sb = pp.tile([CR, BL], f32, tag="h_sb", name="h_sb")
        nc.scalar.activation(h_sb, g1p, AF.Relu, bias=bi1c, scale=sc1c)

        o_p = mlpp.tile([BL, C], f32, tag="o_p", name="o_p")
        nc.tensor.matmul(o_p, h_sb, w2t, start=True, stop=True)

        osc = pp.tile([BL, C], f32, tag="osc", name="osc")
        nc.vector.tensor_mul(osc, o_p, k2)
        ofin = pp.tile([BL, C], f32, tag="ofin", name="ofin")
        nc.vector.tensor_add(ofin, osc, bi2)
        nc.sync.dma_start(out_d[:, :], ofin)

    nc.compile()
    return nc


def _get_nc():
    if "nc" not in _CACHE:
        _CACHE["nc"] = _build_nc()
    return _CACHE["nc"]


def _in_maps(inputs):
    x = np.ascontiguousarray(np.asarray(inputs["x"], dtype=np.float32))
    w1 = np.ascontiguousarray(np.asarray(inputs["w1"], dtype=np.float32))
    w2 = np.ascontiguousarray(np.asarray(inputs["w2"], dtype=np.float32))
    p1 = {
        k: np.ascontiguousarray(
            np.asarray(inputs[k], dtype=np.float32).reshape(CR, 1)
        )
        for k in ("gamma1", "beta1", "mean1", "var1")
    }
    p2 = {
        k: np.ascontiguousarray(np.asarray(inputs[k], dtype=np.float32).reshape(C))
        for k in ("gamma2", "beta2", "mean2", "var2")
    }
    maps = []
    for c in range(NCORES):
        shard = np.ascontiguousarray(x[c * BL : (c + 1) * BL]).reshape(NPAIR, 128, PFD)
        maps.append({"x": shard, "w1": w1, "w2": w2, **p1, **p2})
    return maps


def _run(inputs, trace=False):
    from concourse.bass_utils import run_bass_kernel_spmd

    nc = _get_nc()
    res = run_bass_kernel_spmd(
        nc, _in_maps(inputs), core_ids=list(range(NCORES)), trace=trace
    )
    out = np.concatenate([res.results[c]["out"] for c in range(NCORES)], axis=0)
    return out.reshape(B, 1, 1, C).astype(np.float32), res


def kernel(**inputs) -> np.ndarray:
    out, _ = _run(inputs, trace=False)
    return out


def kernel_traced(**inputs):
    """Returns (out, BassKernelResults) with NTFF profiling enabled."""
    return _run(inputs, trace=True)


def bench(inputs, iters=30, warmup=5):
    """Time the per-step NEFF execution with device-resident inputs.

    Returns (out_full, per_call_seconds_list). Inputs are device_put once;
    each timed call only dispatches the compiled executable, so steady-state
    per-call wall time ~= max-core NEFF exec + dispatch overhead.
    """
    import time
    import jax
    import jax.numpy as jnp
    from jax.sharding import Mesh, PartitionSpec, NamedSharding
    from jax.experimental.shard_map import shard_map
    from concourse import bass2jax, mybir

    bass2jax.install_neuronx_cc_hook()
    nc = _get_nc()

    partition_name = nc.partition_id_tensor.name if nc.partition_id_tensor else None
    in_names, out_names, out_avals = [], [], []
    for alloc in nc.m.functions[0].allocations:
        if not isinstance(alloc, mybir.MemoryLocationSet):
            continue
        name = alloc.memorylocations[0].name
        if alloc.kind == "ExternalInput":
            if name != partition_name:
                in_names.append(name)
        elif alloc.kind == "ExternalOutput":
            out_names.append(name)
            out_avals.append(
                jax.core.ShapedArray(tuple(alloc.tensor_shape), mybir.dt.np(alloc.dtype))
            )
    all_in_names = in_names + out_names
    if partition_name is not None:
        all_in_names = all_in_names + [partition_name]

    def _body(*operands):
        operands = list(operands)
        if partition_name is not None:
            operands.append(bass2jax.partition_id_tensor())
        outs = bass2jax._bass_exec_p.bind(
            *operands,
            out_avals=tuple(out_avals),
            in_names=tuple(all_in_names),
            out_names=tuple(out_names),
            lowering_input_output_aliases=(),
            sim_require_finite=True,
            sim_require_nnan=True,
            nc=nc,
        )
        return tuple(outs)

    devices = jax.devices()[:NCORES]
    mesh = Mesh(np.asarray(devices), ("core",))
    spec = PartitionSpec("core")
    maps = _in_maps(inputs)
    concat = [
        np.concatenate([maps[c][n] for c in range(NCORES)], axis=0) for n in in_names
    ]
    concat += [
        np.zeros((NCORES * a.shape[0], *a.shape[1:]), a.dtype) for a in out_avals
    ]
    sharding = NamedSharding(mesh, spec)
    dev_in = [jax.device_put(a, sharding) for a in concat]

    fn = jax.jit(
        shard_map(
            _body,
            mesh=mesh,
            in_specs=(spec,) * len(concat),
            out_specs=(spec,) * len(out_names),
            check_rep=False,
        )
    )

    for _ in range(warmup):
        outs = fn(*dev_in)
    jax.block_until_ready(outs)

    times = []
    for _ in range(iters):
        t0 = time.perf_counter()
        outs = fn(*dev_in)
        jax.block_until_ready(outs)
        times.append(time.perf_counter() - t0)

    oidx = out_names.index("out")
    o = np.asarray(outs[oidx]).reshape(NCORES, BL, C).reshape(B, C)
    return o.reshape(B, 1, 1, C).astype(np.float32), times
